# revision 52
# baseline (speedup 1.0000x reference)
"""GQA attention (B=2,S=2048,DIM=2048,H=32,KVH=8,HD=64) + RoPE, causal.

Distributed over 8 TRN2 NeuronCores: core = 4*batch + head_group.
Each core computes attention for its 8 q-heads (2 kv-heads) of one batch.
Q^T / K^T are produced directly by the projection matmuls (weights
stationary, x^T moving) so no transpose of Q/K is ever needed; RoPE is
applied in the transposed [hd, seq] layout with replicated cos/sin rows.
The causal mask is fused into the score matmul as an accumulated
(identity x lower-triangular -240) product.  The output projection is
computed per chunk as partial products against the core's own 512 rows
of wo, then summed + distributed with a per-chunk ReduceScatter.
Host-side work is layout-only: weight column/row permutations, batch
split, cos/sin row replication, and concatenation of per-core outputs.
"""
import numpy as np

import concourse.bass as bass
import concourse.bacc as bacc
import concourse.tile as tile
from concourse.tile import add_dep_helper
import concourse.mybir as mybir
from concourse import bass_utils


def _ensure_axon_hooks_shim():
    """bass_utils imports antenv.axon_hooks when BASS_TRACE is set; the
    module is absent in some images. Provide a no-op shim so tracing env
    vars cannot crash the run."""
    import sys, types
    try:
        import antenv  # noqa
        if "antenv.axon_hooks" in sys.modules:
            return
        import importlib
        try:
            importlib.import_module("antenv.axon_hooks")
            return
        except ImportError:
            pass
        mod = types.ModuleType("antenv.axon_hooks")
        mod._hook = None
        mod.get_axon_ntff_profile_hook = lambda: mod._hook

        def set_axon_ntff_profile_hook(h):
            mod._hook = h
        mod.set_axon_ntff_profile_hook = set_axon_ntff_profile_hook
        sys.modules["antenv.axon_hooks"] = mod
        antenv.axon_hooks = mod
    except Exception:
        pass


_ensure_axon_hooks_shim()

F32 = mybir.dt.float32
BF16 = mybir.dt.bfloat16

B, S, DIM = 2, 2048, 2048
H, KVH, HD = 32, 8, 64
N_CORES = 8
GROUPS = [[0, 1, 2, 3], [4, 5, 6, 7]]
NCH = 4            # sequence chunks (queries) of 512
CHUNK = S // NCH   # 512
SEQT = S // 128    # 16 seq tiles
DT = DIM // 128    # 16 contraction tiles
# q-head slot order inside a core: slot s holds local q-head s//2 + 4*(s%2),
# so slot parity == local kv-head index (kv = local_head // 4).
SLOT_TO_LOCAL = [s // 2 + 4 * (s % 2) for s in range(8)]
# rope pair permutation within one head: 16-interleaved halves so the
# (a, b) cross-swap is a within-32-quadrant partition shuffle:
# [a0..a15, b0..b15, a16..a31, b16..b31] where a_i = dim 2i, b_i = dim 2i+1
HD_PERM = np.concatenate([np.arange(0, 32, 2), np.arange(1, 32, 2),
                          np.arange(32, 64, 2), np.arange(33, 64, 2)])
SWAP_MASK = list(range(16, 32)) + list(range(0, 16))
MASK_NEG = -240.0


def _build():
    nc = bacc.Bacc("TRN2", target_bir_lowering=False, debug=False,
                   num_devices=N_CORES)
    x_d = nc.dram_tensor("x", [S, DIM], F32, kind="ExternalInput")
    wq_d = nc.dram_tensor("wq", [DIM, 512], F32, kind="ExternalInput")
    wkv_d = nc.dram_tensor("wkv", [DIM, 256], F32, kind="ExternalInput")
    wo_d = nc.dram_tensor("wo", [512, DIM], F32, kind="ExternalInput")
    cosr_d = nc.dram_tensor("cosr", [128, S], F32, kind="ExternalInput")
    sinr_d = nc.dram_tensor("sinr", [128, S], F32, kind="ExternalInput")
    out_d = nc.dram_tensor("out", [CHUNK, DIM], BF16, kind="ExternalOutput")

    Exp = mybir.ActivationFunctionType.Exp
    Copy = mybir.ActivationFunctionType.Copy

    with tile.TileContext(nc) as tc:
        with tc.tile_pool(name="dram", bufs=1, space="DRAM") as dram, \
             tc.tile_pool(name="wpool", bufs=1) as wpool:
            # ---- DRAM scratch ----
            partial = dram.tile([NCH, CHUNK, DIM], BF16)
            rsout = dram.tile([NCH, 128, DIM], BF16)

            # ---- persistent SBUF ----
            wq_sb = wpool.tile([128, DT, 512], BF16)
            wkv_sb = wpool.tile([128, DT, 256], BF16)
            wo_sb = wpool.tile([128, 4, DIM], BF16)
            cosr_sb = wpool.tile([128, S], BF16)
            sinr_sb = wpool.tile([128, S], BF16)
            kt_sb = wpool.tile([128, S], BF16)        # K^T (kv0|kv1) full seq
            v_sb = wpool.tile([128, SEQT, 130], BF16)  # [V0|1|V1|1] per key tile
            iden_sb = wpool.tile([128, 128], BF16)     # identity
            idenf_sb = wpool.tile([128, 128], F32)     # identity (f32)
            ltneg_sb = wpool.tile([128, 128], BF16)    # MASK_NEG strictly lower

            # constants: ones columns of V_aug; identity; lower-tri mask
            nc.gpsimd.memset(v_sb[:, :, 64:65], 1.0)
            nc.gpsimd.memset(v_sb[:, :, 129:130], 1.0)
            for it in (iden_sb, idenf_sb):
                nc.gpsimd.memset(it[:], 1.0)
                nc.gpsimd.affine_select(
                    out=it[:], in_=it[:],
                    compare_op=mybir.AluOpType.is_equal,
                    fill=0.0, base=0,
                    pattern=[[-1, 128]], channel_multiplier=1,
                )
            nc.gpsimd.memset(ltneg_sb[:], MASK_NEG)
            nc.gpsimd.affine_select(
                out=ltneg_sb[:], in_=ltneg_sb[:],
                compare_op=mybir.AluOpType.is_ge,
                fill=0.0, base=-1,
                pattern=[[-1, 128]], channel_multiplier=1,
            )

            e_sb = wpool.tile([8, 512], BF16)          # recip expand indicator
            nc.gpsimd.memset(e_sb[:], 1.0)
            nc.gpsimd.affine_select(
                out=e_sb[:].rearrange("p (s j) -> p s j", s=8),
                in_=e_sb[:].rearrange("p (s j) -> p s j", s=8),
                compare_op=mybir.AluOpType.is_equal,
                fill=0.0, base=0,
                pattern=[[-1, 8], [0, 64]], channel_multiplier=1,
            )

            e2_sb = wpool.tile([2, 128], BF16)         # 2-row expand indicator
            nc.gpsimd.memset(e2_sb[:], 1.0)
            nc.gpsimd.affine_select(
                out=e2_sb[:].rearrange("p (j i) -> p j i", j=2),
                in_=e2_sb[:].rearrange("p (j i) -> p j i", j=2),
                compare_op=mybir.AluOpType.is_equal,
                fill=0.0, base=0,
                pattern=[[-1, 2], [0, 64]], channel_multiplier=1,
            )

            # preload the exp table set so it doesn't stall the first QK
            warm = wpool.tile([128, 1], F32)
            nc.gpsimd.memset(warm[:], 0.0)
            nc.scalar.activation(warm[:], warm[:], Exp)

            with tc.tile_pool(name="xio", bufs=2) as xio, \
                 tc.tile_pool(name="asb", bufs=2) as asb, \
                 tc.tile_pool(name="bgps", bufs=2, space="PSUM") as bgps, \
                 tc.tile_pool(name="apsum", bufs=1, space="PSUM") as apsum:

                def load_weights():
                    """weights + rope tables: big strided DMAs split over the
                    scalar and sync queues (both idle early)."""
                    rts = []
                    for r in range(2):
                        rt = xio.tile([128, DIM], F32, tag="xf", bufs=3,
                                      name="rt")
                        nc.sync.dma_start(
                            rt[:], (cosr_d if r == 0 else sinr_d)[:])
                        rts.append(rt)
                    for q in range(4):
                        wt = xio.tile([128, 4, 768], F32, tag="wf", bufs=3,
                                      name="wt")
                        kt0 = 4 * q
                        eng = nc.scalar if q % 2 == 0 else nc.sync
                        eng.dma_start(
                            wt[:, :, 0:512],
                            wq_d[kt0 * 128:(kt0 + 4) * 128, :].rearrange(
                                "(k p) n -> p k n", p=128))
                        eng.dma_start(
                            wt[:, :, 512:768],
                            wkv_d[kt0 * 128:(kt0 + 4) * 128, :].rearrange(
                                "(k p) n -> p k n", p=128))
                        nc.scalar.activation(wq_sb[:, kt0:kt0 + 4, :],
                                             wt[:, :, 0:512], Copy)
                        nc.scalar.activation(wkv_sb[:, kt0:kt0 + 4, :],
                                             wt[:, :, 512:768], Copy)
                    for r in range(2):
                        nc.scalar.activation(
                            (cosr_sb if r == 0 else sinr_sb)[:], rts[r][:],
                            Copy)
                    for h in range(4):
                        wof = xio.tile([128, DIM], F32, tag="wof", bufs=1,
                                       name="wof")
                        nc.sync.dma_start(
                            wof[:], wo_d[h * 128:(h + 1) * 128, :])
                        nc.scalar.activation(wo_sb[:, h, :], wof[:], Copy)

                def stage_x(c):
                    """x chunk c: 4 seq-tile f32 loads (gpsimd); the PE
                    transposes them (emitted separately), evacs cast bf16."""
                    xfs = []
                    for tt in range(4):
                        gt = 4 * c + tt
                        xf = xio.tile([128, DIM], F32, tag="xf", bufs=3,
                                      name="xf")
                        nc.gpsimd.dma_start(xf[:], x_d[gt * 128:(gt + 1) * 128, :])
                        xfs.append(xf)
                    xT = xio.tile([128, DT, CHUNK], BF16, tag="xT", bufs=2,
                                  name="xT")
                    return xfs, xT

                def pe_transpose(xfs, xT, tt):
                    """transpose seq-tile tt of a staged chunk into xT via the
                    tensor engine (16 [128,128] f32 transposes, 4 evac-casts)."""
                    for g in range(4):
                        tps = bgps.tile([128, 512], F32, tag="bg", bufs=2,
                                        name="tps")
                        for i in range(4):
                            dt = 4 * g + i
                            nc.tensor.transpose(
                                tps[:, 128 * i:128 * (i + 1)],
                                xfs[tt][:, dt * 128:(dt + 1) * 128],
                                idenf_sb[:])
                        nc.vector.tensor_copy(
                            xT[:, 4 * g:4 * (g + 1), tt * 128:(tt + 1) * 128],
                            tps[:].rearrange("p (a q) -> p a q", a=4))

                def rope(c, ps, out):
                    """ps: [128, 512] f32 PSUM (per 32-quadrant: rows 0:16 = a,
                    16:32 = b); out: [128, 512] bf16 SBUF slice."""
                    cw = slice(c * CHUNK, (c + 1) * CHUNK)
                    t1 = asb.tile([128, CHUNK], BF16, tag="t1", bufs=1,
                                  name="t1")
                    psw = asb.tile([128, CHUNK], F32, tag="psw", bufs=1,
                                   name="psw")
                    t2 = asb.tile([128, CHUNK], BF16, tag="t2", bufs=1,
                                  name="t2")
                    nc.vector.tensor_mul(t1[:], ps[:], cosr_sb[:, cw])
                    nc.vector.stream_shuffle(psw[:], ps[:], SWAP_MASK)
                    nc.vector.tensor_mul(t2[:], psw[:], sinr_sb[:, cw])
                    nc.vector.tensor_add(out, t1[:], t2[:])

                def proj_qt(c, xT, qt, sp):
                    qps = bgps.tile([128, CHUNK], F32, tag="bg", bufs=2,
                                    name="qps")
                    for dt in range(DT):
                        nc.tensor.matmul(
                            qps[:], wq_sb[:, dt, sp * 128:(sp + 1) * 128],
                            xT[:, dt, :], start=(dt == 0), stop=(dt == DT - 1))
                    rope(c, qps, qt[:, sp, :])

                def proj_kv(c, xT):
                    kps = bgps.tile([128, CHUNK], F32, tag="bg", bufs=2,
                                    name="kps")
                    for dt in range(DT):
                        nc.tensor.matmul(
                            kps[:], wkv_sb[:, dt, 0:128],
                            xT[:, dt, :], start=(dt == 0), stop=(dt == DT - 1))
                    rope(c, kps, kt_sb[:, c * CHUNK:(c + 1) * CHUNK])
                    vps = bgps.tile([128, CHUNK], F32, tag="bg", bufs=2,
                                    name="vps")
                    for tt in range(4):
                        gt = 4 * c + tt
                        for dt in range(DT):
                            nc.tensor.matmul(
                                vps[:, tt * 128:(tt + 1) * 128],
                                xT[:, dt, tt * 128:(tt + 1) * 128],
                                wkv_sb[:, dt, 128:256],
                                start=(dt == 0), stop=(dt == DT - 1))
                        nc.vector.tensor_copy(v_sb[:, gt, 0:64],
                                              vps[:, tt * 128:tt * 128 + 64])
                        nc.vector.tensor_copy(v_sb[:, gt, 65:129],
                                              vps[:, tt * 128 + 64:tt * 128 + 128])

                def emit_scale(pc, pstages, pdenoms, tail=False):
                    """normalize stages directly into the stacked wo
                    stationary (DVE writes partition-shifted for slot j=1)."""
                    recipf = asb.tile([8, CHUNK], F32, tag="recipf", bufs=1,
                                      name="recipf")
                    nc.vector.tensor_copy(recipf[:], pdenoms[:])
                    recip8 = asb.tile([8, CHUNK], F32, tag="recip", bufs=1,
                                      name="recip8")
                    nc.vector.reciprocal_approx_fast(recip8[:], recipf[:])
                    precipb = asb.tile([8, CHUNK], BF16, tag="recipb", bufs=1,
                                       name="recip8b")
                    nc.vector.tensor_copy(precipb[:], recip8[:])
                    sts = asb.tile([128, 4, CHUNK], BF16, tag="sts", bufs=2,
                                   name="sts")
                    for sp in range(4):
                        for j in range(2):
                            s = 2 * sp + j
                            rexp = bgps.tile([128, 512], F32, tag="bg",
                                             bufs=2, name="rexp")
                            nc.tensor.matmul(
                                rexp[0:64, :],
                                e_sb[:, 64 * s:64 * (s + 1)], precipb[:],
                                start=True, stop=True)
                            nc.vector.tensor_mul(
                                sts[64 * j:64 * (j + 1), sp, :],
                                pstages[sp][0:64, 512 * j:512 * (j + 1)],
                                rexp[0:64, :])
                    return sts

                def emit_wo(pc, sts, qs_list, tail=False):
                    pools = ([("bg", bgps), ("sps", apsum), ("aps", apsum)]
                             if tail else [("bg", bgps)])
                    gi = 0
                    for qs in qs_list:
                        for nb in range(4):
                            tag, pool = pools[gi % len(pools)]
                            gi += 1
                            wop = pool.tile(
                                [128, 512 if tag == "bg" else 1024], F32,
                                tag=tag, bufs=2 if tag != "aps" else 1,
                                name="wop")
                            for sp in range(4):
                                nc.tensor.matmul(
                                    wop[:, 0:512],
                                    sts[:, sp, qs * 128:(qs + 1) * 128],
                                    wo_sb[:, sp, nb * 512:(nb + 1) * 512],
                                    start=(sp == 0), stop=(sp == 3))
                            ostage = asb.tile([128, 512], BF16, tag="ost",
                                              bufs=2, name="ostage")
                            nc.vector.tensor_copy(ostage[:], wop[:, 0:512])
                            peng = nc.scalar if (tail and nb % 2 == 0) else nc.sync
                            pdmas[pc].append(peng.dma_start(
                                partial[pc, qs * 128:(qs + 1) * 128,
                                        nb * 512:(nb + 1) * 512], ostage[:]))

                def emit_cc(pc):
                    cc = nc.gpsimd.collective_compute(
                        "ReduceScatter", mybir.AluOpType.add,
                        replica_groups=GROUPS,
                        ins=[partial[pc][:, :].opt()],
                        outs=[rsout[pc][:, :].opt()])
                    for d in pdmas[pc]:
                        add_dep_helper(cc.ins, d.ins, sync=True,
                                       reason="RS waits partial DMAs")
                    cc_insts.append(cc)
                    od = nc.gpsimd.dma_start(
                        out_d[pc * 128:(pc + 1) * 128, :], rsout[pc][:, :])
                    add_dep_helper(od.ins, cc.ins, sync=True,
                                   reason="out copy waits RS")

                # ---- fused main loop ----
                cc_insts = []
                pdmas = [[] for _ in range(NCH)]
                xbs, xT = stage_x(0)
                for tt in range(4):
                    pe_transpose(xbs, xT, tt)
                load_weights()
                pending = None
                psts = {}
                nxt = None

                def attn_kts(c, qt, sp, aps, kt_lo, kt_hi):
                    for kt in range(kt_lo, kt_hi):
                        vs = max(0, 128 * kt - CHUNK * c)
                        diag = kt >= 4 * c
                        spt = apsum.tile([128, 1024], F32, tag="sps",
                                         bufs=2, name="spt")
                        for j in range(2):
                            nc.tensor.matmul(
                                spt[:, 512 * j + vs:512 * j + 512],
                                kt_sb[64 * j:64 * j + 64, kt * 128:(kt + 1) * 128],
                                qt[64 * j:64 * j + 64, sp, vs:CHUNK],
                                start=True, stop=not diag)
                        if diag:
                            for j in range(2):
                                nc.tensor.matmul(
                                    spt[:, 512 * j + vs:512 * j + vs + 128],
                                    iden_sb[:], ltneg_sb[:],
                                    start=False, stop=True,
                                    skip_group_check=True)
                        pt = asb.tile([128, 1024], BF16, tag="pT", bufs=3,
                                      name="pt")
                        nc.scalar.activation(
                            pt[:].rearrange("p (h q) -> p h q", h=2)[:, :, vs:512],
                            spt[:].rearrange("p (h q) -> p h q", h=2)[:, :, vs:512],
                            Exp, scale=0.125)
                        for j in range(2):
                            nc.tensor.matmul(
                                aps[0:65, 512 * j + vs:512 * j + 512],
                                v_sb[:, kt, 65 * j:65 * j + 65],
                                pt[:, 512 * j + vs:512 * j + 512],
                                start=(kt == 0), stop=(kt == 4 * c + 3))

                qt = xio.tile([128, 4, CHUNK], BF16, tag="qt", bufs=2,
                              name="qt")
                proj_qt(0, xT, qt, 0)
                proj_kv(0, xT)
                for s2 in (1, 2, 3):
                    proj_qt(0, xT, qt, s2)
                for c in range(NCH):
                    last = c == NCH - 1
                    if c + 1 < NCH:
                        nxt = stage_x(c + 1)
                    denoms8 = asb.tile([8, CHUNK], BF16, tag="denoms", bufs=1,
                                       name="denoms8")
                    stgs = []
                    if last:
                        sts3 = asb.tile([128, 4, CHUNK], BF16, tag="sts",
                                        bufs=2, name="sts3")
                    qt_next = None
                    for sp in range(4):
                        aps = apsum.tile([128, 1024], F32, tag="aps", bufs=1,
                                         name="aps")
                        attn_kts(c, qt, sp, aps, 0, 4 * c + 4)
                        stg = asb.tile([128, 1024], BF16, tag="stage", bufs=4,
                                       name="stg")
                        nc.vector.tensor_copy(stg[0:65, :], aps[0:65, :])
                        if last:
                            denoms2 = asb.tile([2, CHUNK], BF16, tag="denoms",
                                               bufs=1, name="denoms2")
                        for j in range(2):
                            s = 2 * sp + j
                            eng = nc.scalar if last else nc.sync
                            eng.dma_start(
                                denoms2[j:j + 1, :] if last
                                else denoms8[s:s + 1, :],
                                stg[64:65, 512 * j:512 * (j + 1)])
                        stgs.append(stg)
                        if pending is not None:
                            ppc = pending[0]
                            if sp == 0:
                                psts[ppc] = emit_scale(*pending)
                            elif sp == 1:
                                emit_wo(ppc, psts[ppc], [0, 1])
                            elif sp == 2:
                                emit_wo(ppc, psts[ppc], [2, 3])
                                emit_cc(ppc)
                                pending = None
                        if last:
                            # inline per-slot-pair normalization: only sp3's
                            # chain remains on the critical tail
                            recip2f = asb.tile([2, CHUNK], F32, tag="recipf",
                                               bufs=1, name="recip2f")
                            nc.vector.tensor_copy(recip2f[:], denoms2[0:2, :])
                            recip2 = asb.tile([2, CHUNK], F32, tag="recip",
                                              bufs=1, name="recip2")
                            nc.vector.reciprocal_approx_fast(recip2[:],
                                                             recip2f[:])
                            precip2 = asb.tile([2, CHUNK], BF16, tag="recipb",
                                               bufs=1, name="precip2")
                            nc.vector.tensor_copy(precip2[:], recip2[:])
                            for j in range(2):
                                rexp = bgps.tile([128, 512], F32, tag="bg",
                                                 bufs=2, name="rexp")
                                nc.tensor.matmul(
                                    rexp[0:64, :],
                                    e2_sb[:, 64 * j:64 * (j + 1)],
                                    precip2[:],
                                    start=True, stop=True)
                                nc.vector.tensor_mul(
                                    sts3[64 * j:64 * (j + 1), sp, :],
                                    stg[0:64, 512 * j:512 * (j + 1)],
                                    rexp[0:64, :])
                        else:
                            if sp == 0:
                                pe_transpose(nxt[0], nxt[1], 0)
                                pe_transpose(nxt[0], nxt[1], 1)
                            elif sp == 1:
                                pe_transpose(nxt[0], nxt[1], 2)
                            elif sp == 2:
                                # pipeline the next chunk's projections into
                                # this chunk's last attention legs
                                pe_transpose(nxt[0], nxt[1], 3)
                                qt_next = xio.tile([128, 4, CHUNK], BF16,
                                                   tag="qt", bufs=2, name="qt")
                                proj_qt(c + 1, nxt[1], qt_next, 0)
                                proj_kv(c + 1, nxt[1])
                                for s2 in (1, 2, 3):
                                    proj_qt(c + 1, nxt[1], qt_next, s2)
                    if last:
                        emit_wo(c, sts3, [0, 1, 2, 3], tail=True)
                        emit_cc(c)
                    else:
                        pending = (c, stgs, denoms8)
                        xT = nxt[1]
                        qt = qt_next

    nc.finalize()
    return nc


_NC_CACHE = None


def _get_nc():
    global _NC_CACHE
    if _NC_CACHE is None:
        _NC_CACHE = _build()
    return _NC_CACHE


def _shard_inputs(x, wq, wk, wv, wo, freqs_cos, freqs_sin):
    """Pure layout work: slice batch, pick each core's heads, permute rope
    pairs within each head, shard wo rows per core, replicate cos/sin."""
    x = np.ascontiguousarray(np.asarray(x, dtype=np.float32))
    wq = np.asarray(wq, dtype=np.float32)
    wk = np.asarray(wk, dtype=np.float32)
    wv = np.asarray(wv, dtype=np.float32)
    wo = np.asarray(wo, dtype=np.float32)
    cos = np.asarray(freqs_cos, dtype=np.float32)
    sin = np.asarray(freqs_sin, dtype=np.float32)

    # replicated rope tables matching the transposed Q^T/K^T row layout:
    # row r (within a 64-row slot block, w = r % 64, quadrant q2 = w // 16):
    # freq index i = (q2 // 2) * 16 + (w % 16); a-halves (q2 even) get -sin.
    cosr = np.empty((128, S), dtype=np.float32)
    sinr = np.empty((128, S), dtype=np.float32)
    for r in range(128):
        w = r % 64
        q2 = w // 16
        i = (q2 // 2) * 16 + (w % 16)
        cosr[r] = cos[:, i]
        sinr[r] = (-1.0 if q2 % 2 == 0 else 1.0) * sin[:, i]
    cosr = np.ascontiguousarray(cosr)
    sinr = np.ascontiguousarray(sinr)

    in_maps = []
    for core in range(N_CORES):
        b, g = core // 4, core % 4
        wq_cols = []
        wo_rows = []
        for s_ in range(8):
            h = 8 * g + SLOT_TO_LOCAL[s_]
            wq_cols.append(wq[:, 64 * h + HD_PERM])
            wo_rows.append(wo[64 * h:64 * (h + 1), :])
        wq_s = np.ascontiguousarray(np.concatenate(wq_cols, axis=1))
        wo_s = np.ascontiguousarray(np.concatenate(wo_rows, axis=0))
        wk_cols = [wk[:, 64 * (2 * g + j) + HD_PERM] for j in range(2)]
        wv_cols = wv[:, 64 * 2 * g: 64 * (2 * g + 2)]
        wkv_s = np.ascontiguousarray(
            np.concatenate(wk_cols + [wv_cols], axis=1))
        in_maps.append({
            "x": x[b], "wq": wq_s, "wkv": wkv_s, "wo": wo_s,
            "cosr": cosr, "sinr": sinr,
        })
    return in_maps


def kernel(x, wq, wk, wv, wo, freqs_cos, freqs_sin, mask=None, start_pos=0,
           **_unused):
    nc = _get_nc()
    in_maps = _shard_inputs(x, wq, wk, wv, wo, freqs_cos, freqs_sin)
    res = bass_utils.run_bass_kernel_spmd(
        nc, in_maps, core_ids=list(range(N_CORES)))
    out = np.empty((B, S, DIM), dtype=np.float32)
    for core in range(N_CORES):
        b, g = core // 4, core % 4
        co = np.asarray(res.results[core]["out"]).astype(np.float32)
        for c in range(NCH):
            out[b, CHUNK * c + 128 * g: CHUNK * c + 128 * (g + 1), :] = \
                co[128 * c:128 * (c + 1), :]
    return out


# revision 53
# speedup vs baseline: 1.0765x; 1.0765x over previous
"""GQA attention (B=2,S=2048,DIM=2048,H=32,KVH=8,HD=64) + RoPE, causal.

Distributed over 8 TRN2 NeuronCores: core = 4*batch + head_group.
Each core computes attention for its 8 q-heads (2 kv-heads) of one batch.
Q^T / K^T are produced directly by the projection matmuls (weights
stationary, x^T moving) so no transpose of Q/K is ever needed; RoPE is
applied in the transposed [hd, seq] layout with replicated cos/sin rows.
The causal mask is fused into the score matmul as an accumulated
(identity x lower-triangular -240) product.  The output projection is
computed per chunk as partial products against the core's own 512 rows
of wo, then summed + distributed with a per-chunk ReduceScatter.
Host-side work is layout-only: weight column/row permutations, batch
split, cos/sin row replication, and concatenation of per-core outputs.
"""
import numpy as np

import concourse.bass as bass
import concourse.bacc as bacc
import concourse.tile as tile
from concourse.tile import add_dep_helper
import concourse.mybir as mybir
from concourse import bass_utils


def _ensure_axon_hooks_shim():
    """bass_utils imports antenv.axon_hooks when BASS_TRACE is set; the
    module is absent in some images. Provide a no-op shim so tracing env
    vars cannot crash the run."""
    import sys, types
    try:
        import antenv  # noqa
        if "antenv.axon_hooks" in sys.modules:
            return
        import importlib
        try:
            importlib.import_module("antenv.axon_hooks")
            return
        except ImportError:
            pass
        mod = types.ModuleType("antenv.axon_hooks")
        mod._hook = None
        mod.get_axon_ntff_profile_hook = lambda: mod._hook

        def set_axon_ntff_profile_hook(h):
            mod._hook = h
        mod.set_axon_ntff_profile_hook = set_axon_ntff_profile_hook
        sys.modules["antenv.axon_hooks"] = mod
        antenv.axon_hooks = mod
    except Exception:
        pass


_ensure_axon_hooks_shim()

F32 = mybir.dt.float32
BF16 = mybir.dt.bfloat16

B, S, DIM = 2, 2048, 2048
H, KVH, HD = 32, 8, 64
N_CORES = 8
GROUPS = [[0, 1, 2, 3], [4, 5, 6, 7]]
NCH = 4            # sequence chunks (queries) of 512
CHUNK = S // NCH   # 512
SEQT = S // 128    # 16 seq tiles
DT = DIM // 128    # 16 contraction tiles
# q-head slot order inside a core: slot s holds local q-head s//2 + 4*(s%2),
# so slot parity == local kv-head index (kv = local_head // 4).
SLOT_TO_LOCAL = [s // 2 + 4 * (s % 2) for s in range(8)]
# rope pair permutation within one head: 16-interleaved halves so the
# (a, b) cross-swap is a within-32-quadrant partition shuffle:
# [a0..a15, b0..b15, a16..a31, b16..b31] where a_i = dim 2i, b_i = dim 2i+1
HD_PERM = np.concatenate([np.arange(0, 32, 2), np.arange(1, 32, 2),
                          np.arange(32, 64, 2), np.arange(33, 64, 2)])
SWAP_MASK = list(range(16, 32)) + list(range(0, 16))
MASK_NEG = -240.0


def _build():
    nc = bacc.Bacc("TRN2", target_bir_lowering=False, debug=False,
                   num_devices=N_CORES)
    x_d = nc.dram_tensor("x", [S, DIM], F32, kind="ExternalInput")
    wq_d = nc.dram_tensor("wq", [DIM, 512], F32, kind="ExternalInput")
    wkv_d = nc.dram_tensor("wkv", [DIM, 256], F32, kind="ExternalInput")
    wo_d = nc.dram_tensor("wo", [512, DIM], F32, kind="ExternalInput")
    cosr_d = nc.dram_tensor("cosr", [128, S], F32, kind="ExternalInput")
    sinr_d = nc.dram_tensor("sinr", [128, S], F32, kind="ExternalInput")
    out_d = nc.dram_tensor("out", [CHUNK, DIM], BF16, kind="ExternalOutput")

    Exp = mybir.ActivationFunctionType.Exp
    Copy = mybir.ActivationFunctionType.Copy

    with tile.TileContext(nc) as tc:
        with tc.tile_pool(name="dram", bufs=1, space="DRAM") as dram, \
             tc.tile_pool(name="wpool", bufs=1) as wpool:
            # ---- DRAM scratch ----
            partial = dram.tile([NCH, CHUNK, DIM], BF16)
            rsout = dram.tile([NCH, 128, DIM], BF16)

            # ---- persistent SBUF ----
            wq_sb = wpool.tile([128, DT, 512], BF16)
            wkv_sb = wpool.tile([128, DT, 256], BF16)
            wo_sb = wpool.tile([128, 4, DIM], BF16)
            cosr_sb = wpool.tile([128, S], BF16)
            sinr_sb = wpool.tile([128, S], BF16)
            kt_sb = wpool.tile([128, S], BF16)        # K^T (kv0|kv1) full seq
            v_sb = wpool.tile([128, SEQT, 130], BF16)  # [V0|1|V1|1] per key tile
            iden_sb = wpool.tile([128, 128], BF16)     # identity
            idenf_sb = wpool.tile([128, 128], F32)     # identity (f32)
            ltneg_sb = wpool.tile([128, 128], BF16)    # MASK_NEG strictly lower

            # constants: ones columns of V_aug; identity; lower-tri mask
            nc.gpsimd.memset(v_sb[:, :, 64:65], 1.0)
            nc.gpsimd.memset(v_sb[:, :, 129:130], 1.0)
            for it in (iden_sb, idenf_sb):
                nc.gpsimd.memset(it[:], 1.0)
                nc.gpsimd.affine_select(
                    out=it[:], in_=it[:],
                    compare_op=mybir.AluOpType.is_equal,
                    fill=0.0, base=0,
                    pattern=[[-1, 128]], channel_multiplier=1,
                )
            nc.gpsimd.memset(ltneg_sb[:], MASK_NEG)
            nc.gpsimd.affine_select(
                out=ltneg_sb[:], in_=ltneg_sb[:],
                compare_op=mybir.AluOpType.is_ge,
                fill=0.0, base=-1,
                pattern=[[-1, 128]], channel_multiplier=1,
            )

            e_sb = wpool.tile([8, 512], BF16)          # recip expand indicator
            nc.gpsimd.memset(e_sb[:], 1.0)
            nc.gpsimd.affine_select(
                out=e_sb[:].rearrange("p (s j) -> p s j", s=8),
                in_=e_sb[:].rearrange("p (s j) -> p s j", s=8),
                compare_op=mybir.AluOpType.is_equal,
                fill=0.0, base=0,
                pattern=[[-1, 8], [0, 64]], channel_multiplier=1,
            )

            e2_sb = wpool.tile([2, 128], BF16)         # 2-row expand indicator
            nc.gpsimd.memset(e2_sb[:], 1.0)
            nc.gpsimd.affine_select(
                out=e2_sb[:].rearrange("p (j i) -> p j i", j=2),
                in_=e2_sb[:].rearrange("p (j i) -> p j i", j=2),
                compare_op=mybir.AluOpType.is_equal,
                fill=0.0, base=0,
                pattern=[[-1, 2], [0, 64]], channel_multiplier=1,
            )

            # preload the exp table set so it doesn't stall the first QK
            warm = wpool.tile([128, 1], F32)
            nc.gpsimd.memset(warm[:], 0.0)
            nc.scalar.activation(warm[:], warm[:], Exp)

            with tc.tile_pool(name="xio", bufs=2) as xio, \
                 tc.tile_pool(name="asb", bufs=2) as asb, \
                 tc.tile_pool(name="bgps", bufs=2, space="PSUM") as bgps, \
                 tc.tile_pool(name="apsum", bufs=1, space="PSUM") as apsum:

                def load_weights():
                    """weights + rope tables: big strided DMAs split over the
                    scalar and sync queues (both idle early)."""
                    rts = []
                    for r in range(2):
                        rt = xio.tile([128, DIM], F32, tag="xf", bufs=3,
                                      name="rt")
                        nc.sync.dma_start(
                            rt[:], (cosr_d if r == 0 else sinr_d)[:])
                        rts.append(rt)
                    for q in range(4):
                        wt = xio.tile([128, 4, 768], F32, tag="wf", bufs=3,
                                      name="wt")
                        kt0 = 4 * q
                        eng = nc.scalar if q % 2 == 0 else nc.sync
                        eng.dma_start(
                            wt[:, :, 0:512],
                            wq_d[kt0 * 128:(kt0 + 4) * 128, :].rearrange(
                                "(k p) n -> p k n", p=128))
                        eng.dma_start(
                            wt[:, :, 512:768],
                            wkv_d[kt0 * 128:(kt0 + 4) * 128, :].rearrange(
                                "(k p) n -> p k n", p=128))
                        nc.scalar.activation(wq_sb[:, kt0:kt0 + 4, :],
                                             wt[:, :, 0:512], Copy)
                        nc.scalar.activation(wkv_sb[:, kt0:kt0 + 4, :],
                                             wt[:, :, 512:768], Copy)
                    for r in range(2):
                        nc.scalar.activation(
                            (cosr_sb if r == 0 else sinr_sb)[:], rts[r][:],
                            Copy)
                    for h in range(4):
                        wof = xio.tile([128, DIM], F32, tag="wof", bufs=1,
                                       name="wof")
                        nc.sync.dma_start(
                            wof[:], wo_d[h * 128:(h + 1) * 128, :])
                        nc.scalar.activation(wo_sb[:, h, :], wof[:], Copy)

                def stage_x(c):
                    """x chunk c: 4 seq-tile f32 loads (gpsimd); the PE
                    transposes them (emitted separately), evacs cast bf16."""
                    xfs = []
                    for tt in range(4):
                        gt = 4 * c + tt
                        xf = xio.tile([128, DIM], F32, tag="xf", bufs=3,
                                      name="xf")
                        nc.gpsimd.dma_start(xf[:], x_d[gt * 128:(gt + 1) * 128, :])
                        xfs.append(xf)
                    xT = xio.tile([128, DT, CHUNK], BF16, tag="xT", bufs=2,
                                  name="xT")
                    return xfs, xT

                def pe_transpose(xfs, xT, tt):
                    """transpose seq-tile tt of a staged chunk into xT via the
                    tensor engine (16 [128,128] f32 transposes, 4 evac-casts)."""
                    for g in range(4):
                        tps = bgps.tile([128, 512], F32, tag="bg", bufs=2,
                                        name="tps")
                        for i in range(4):
                            dt = 4 * g + i
                            nc.tensor.transpose(
                                tps[:, 128 * i:128 * (i + 1)],
                                xfs[tt][:, dt * 128:(dt + 1) * 128],
                                idenf_sb[:])
                        nc.vector.tensor_copy(
                            xT[:, 4 * g:4 * (g + 1), tt * 128:(tt + 1) * 128],
                            tps[:].rearrange("p (a q) -> p a q", a=4))

                def rope(c, ps, out):
                    """ps: [128, 512] f32 PSUM (per 32-quadrant: rows 0:16 = a,
                    16:32 = b); out: [128, 512] bf16 SBUF slice."""
                    cw = slice(c * CHUNK, (c + 1) * CHUNK)
                    t1 = asb.tile([128, CHUNK], BF16, tag="t1", bufs=1,
                                  name="t1")
                    psw = asb.tile([128, CHUNK], F32, tag="psw", bufs=1,
                                   name="psw")
                    t2 = asb.tile([128, CHUNK], BF16, tag="t2", bufs=1,
                                  name="t2")
                    nc.vector.tensor_mul(t1[:], ps[:], cosr_sb[:, cw])
                    nc.vector.stream_shuffle(psw[:], ps[:], SWAP_MASK)
                    nc.vector.tensor_mul(t2[:], psw[:], sinr_sb[:, cw])
                    nc.vector.tensor_add(out, t1[:], t2[:])

                def proj_qt(c, xT, qt, sp):
                    qps = bgps.tile([128, CHUNK], F32, tag="bg", bufs=2,
                                    name="qps")
                    for dt in range(DT):
                        nc.tensor.matmul(
                            qps[:], wq_sb[:, dt, sp * 128:(sp + 1) * 128],
                            xT[:, dt, :], start=(dt == 0), stop=(dt == DT - 1))
                    rope(c, qps, qt[:, sp, :])

                def proj_kv(c, xT):
                    kps = bgps.tile([128, CHUNK], F32, tag="bg", bufs=2,
                                    name="kps")
                    for dt in range(DT):
                        nc.tensor.matmul(
                            kps[:], wkv_sb[:, dt, 0:128],
                            xT[:, dt, :], start=(dt == 0), stop=(dt == DT - 1))
                    rope(c, kps, kt_sb[:, c * CHUNK:(c + 1) * CHUNK])
                    vps = bgps.tile([128, CHUNK], F32, tag="bg", bufs=2,
                                    name="vps")
                    for tt in range(4):
                        gt = 4 * c + tt
                        for dt in range(DT):
                            nc.tensor.matmul(
                                vps[:, tt * 128:(tt + 1) * 128],
                                xT[:, dt, tt * 128:(tt + 1) * 128],
                                wkv_sb[:, dt, 128:256],
                                start=(dt == 0), stop=(dt == DT - 1))
                        nc.vector.tensor_copy(v_sb[:, gt, 0:64],
                                              vps[:, tt * 128:tt * 128 + 64])
                        nc.vector.tensor_copy(v_sb[:, gt, 65:129],
                                              vps[:, tt * 128 + 64:tt * 128 + 128])

                def emit_scale(pc, pstages, pdenoms, tail=False):
                    """normalize stages directly into the stacked wo
                    stationary (DVE writes partition-shifted for slot j=1)."""
                    recipf = asb.tile([8, CHUNK], F32, tag="recipf", bufs=1,
                                      name="recipf")
                    nc.vector.tensor_copy(recipf[:], pdenoms[:])
                    recip8 = asb.tile([8, CHUNK], F32, tag="recip", bufs=1,
                                      name="recip8")
                    nc.vector.reciprocal_approx_fast(recip8[:], recipf[:])
                    precipb = asb.tile([8, CHUNK], BF16, tag="recipb", bufs=1,
                                       name="recip8b")
                    nc.vector.tensor_copy(precipb[:], recip8[:])
                    sts = asb.tile([128, 4, CHUNK], BF16, tag="sts", bufs=2,
                                   name="sts")
                    for sp in range(4):
                        for j in range(2):
                            s = 2 * sp + j
                            rexp = bgps.tile([128, 512], F32, tag="bg",
                                             bufs=2, name="rexp")
                            nc.tensor.matmul(
                                rexp[0:64, :],
                                e_sb[:, 64 * s:64 * (s + 1)], precipb[:],
                                start=True, stop=True)
                            nc.vector.tensor_mul(
                                sts[64 * j:64 * (j + 1), sp, :],
                                pstages[sp][0:64, 512 * j:512 * (j + 1)],
                                rexp[0:64, :])
                    return sts

                def emit_wo(pc, sts, qs_list, tail=False):
                    pools = ([("bg", bgps), ("sps", apsum), ("aps", apsum)]
                             if tail else [("bg", bgps)])
                    gi = 0
                    for qs in qs_list:
                        for nb in range(4):
                            tag, pool = pools[gi % len(pools)]
                            gi += 1
                            wop = pool.tile(
                                [128, 512 if tag == "bg" else 1024], F32,
                                tag=tag, bufs=2 if tag != "aps" else 1,
                                name="wop")
                            for sp in range(4):
                                nc.tensor.matmul(
                                    wop[:, 0:512],
                                    sts[:, sp, qs * 128:(qs + 1) * 128],
                                    wo_sb[:, sp, nb * 512:(nb + 1) * 512],
                                    start=(sp == 0), stop=(sp == 3))
                            ostage = asb.tile([128, 512], BF16, tag="ost",
                                              bufs=2, name="ostage")
                            nc.vector.tensor_copy(ostage[:], wop[:, 0:512])
                            peng = nc.scalar if (tail and nb % 2 == 0) else nc.sync
                            pdmas[pc].append(peng.dma_start(
                                partial[pc, qs * 128:(qs + 1) * 128,
                                        nb * 512:(nb + 1) * 512], ostage[:]))

                def emit_cc(pc):
                    cc = nc.gpsimd.collective_compute(
                        "ReduceScatter", mybir.AluOpType.add,
                        replica_groups=GROUPS,
                        ins=[partial[pc][:, :].opt()],
                        outs=[rsout[pc][:, :].opt()])
                    for d in pdmas[pc]:
                        add_dep_helper(cc.ins, d.ins, sync=True,
                                       reason="RS waits partial DMAs")
                    cc_insts.append(cc)
                    od = nc.gpsimd.dma_start(
                        out_d[pc * 128:(pc + 1) * 128, :], rsout[pc][:, :])
                    add_dep_helper(od.ins, cc.ins, sync=True,
                                   reason="out copy waits RS")

                # ---- fused main loop ----
                cc_insts = []
                pdmas = [[] for _ in range(NCH)]
                xbs, xT = stage_x(0)
                for tt in range(4):
                    pe_transpose(xbs, xT, tt)
                load_weights()
                pending = None
                psts = {}
                nxt = None

                def attn_kts(c, qt, sp, aps, kt_lo, kt_hi):
                    for kt in range(kt_lo, kt_hi):
                        vs = max(0, 128 * kt - CHUNK * c)
                        diag = kt >= 4 * c
                        spt = apsum.tile([128, 1024], F32, tag="sps",
                                         bufs=2, name="spt")
                        for j in range(2):
                            nc.tensor.matmul(
                                spt[:, 512 * j + vs:512 * j + 512],
                                kt_sb[64 * j:64 * j + 64, kt * 128:(kt + 1) * 128],
                                qt[64 * j:64 * j + 64, sp, vs:CHUNK],
                                start=True, stop=not diag)
                        if diag:
                            for j in range(2):
                                nc.tensor.matmul(
                                    spt[:, 512 * j + vs:512 * j + vs + 128],
                                    iden_sb[:], ltneg_sb[:],
                                    start=False, stop=True,
                                    skip_group_check=True)
                        pt = asb.tile([128, 1024], BF16, tag="pT", bufs=3,
                                      name="pt")
                        nc.scalar.activation(
                            pt[:].rearrange("p (h q) -> p h q", h=2)[:, :, vs:512],
                            spt[:].rearrange("p (h q) -> p h q", h=2)[:, :, vs:512],
                            Exp, scale=0.125)
                        for j in range(2):
                            nc.tensor.matmul(
                                aps[0:65, 512 * j + vs:512 * j + 512],
                                v_sb[:, kt, 65 * j:65 * j + 65],
                                pt[:, 512 * j + vs:512 * j + 512],
                                start=(kt == 0), stop=(kt == 4 * c + 3))

                qt = xio.tile([128, 4, CHUNK], BF16, tag="qt", bufs=2,
                              name="qt")
                proj_qt(0, xT, qt, 0)
                proj_kv(0, xT)
                for s2 in (1, 2, 3):
                    proj_qt(0, xT, qt, s2)
                for c in range(NCH):
                    last = c == NCH - 1
                    if c + 1 < NCH:
                        nxt = stage_x(c + 1)
                    denoms8 = asb.tile([8, CHUNK], BF16, tag="denoms", bufs=1,
                                       name="denoms8")
                    stgs = []
                    if last:
                        sts3 = asb.tile([128, 4, CHUNK], BF16, tag="sts",
                                        bufs=2, name="sts3")
                    qt_next = None
                    for sp in range(4):
                        aps = apsum.tile([128, 1024], F32, tag="aps", bufs=1,
                                         name="aps")
                        attn_kts(c, qt, sp, aps, 0, 4 * c + 4)
                        stg = asb.tile([128, 1024], BF16, tag="stage", bufs=4,
                                       name="stg")
                        nc.vector.tensor_copy(stg[0:65, :], aps[0:65, :])
                        if last:
                            denoms2 = asb.tile([2, CHUNK], BF16, tag="denoms",
                                               bufs=1, name="denoms2")
                        for j in range(2):
                            s = 2 * sp + j
                            eng = nc.scalar if last else nc.sync
                            eng.dma_start(
                                denoms2[j:j + 1, :] if last
                                else denoms8[s:s + 1, :],
                                stg[64:65, 512 * j:512 * (j + 1)])
                        stgs.append(stg)
                        if pending is not None:
                            ppc = pending[0]
                            if sp == 0:
                                psts[ppc] = emit_scale(*pending)
                            elif sp == 1:
                                emit_wo(ppc, psts[ppc], [0, 1])
                            elif sp == 2:
                                emit_wo(ppc, psts[ppc], [2, 3])
                                emit_cc(ppc)
                                pending = None
                        if last:
                            # inline per-slot-pair normalization: only sp3's
                            # chain remains on the critical tail
                            recip2f = asb.tile([2, CHUNK], F32, tag="recipf",
                                               bufs=1, name="recip2f")
                            nc.vector.tensor_copy(recip2f[:], denoms2[0:2, :])
                            recip2 = asb.tile([2, CHUNK], F32, tag="recip",
                                              bufs=1, name="recip2")
                            nc.vector.reciprocal_approx_fast(recip2[:],
                                                             recip2f[:])
                            precip2 = asb.tile([2, CHUNK], BF16, tag="recipb",
                                               bufs=1, name="precip2")
                            nc.vector.tensor_copy(precip2[:], recip2[:])
                            for j in range(2):
                                rexp = bgps.tile([128, 512], F32, tag="bg",
                                                 bufs=2, name="rexp")
                                nc.tensor.matmul(
                                    rexp[0:64, :],
                                    e2_sb[:, 64 * j:64 * (j + 1)],
                                    precip2[:],
                                    start=True, stop=True)
                                nc.vector.tensor_mul(
                                    sts3[64 * j:64 * (j + 1), sp, :],
                                    stg[0:64, 512 * j:512 * (j + 1)],
                                    rexp[0:64, :])
                        else:
                            if sp == 0:
                                pe_transpose(nxt[0], nxt[1], 0)
                                pe_transpose(nxt[0], nxt[1], 1)
                            elif sp == 1:
                                pe_transpose(nxt[0], nxt[1], 2)
                                pe_transpose(nxt[0], nxt[1], 3)
                            elif sp == 3:
                                # pipeline the next chunk's projections into
                                # this chunk's last attention leg
                                qt_next = xio.tile([128, 4, CHUNK], BF16,
                                                   tag="qt", bufs=2, name="qt")
                                proj_qt(c + 1, nxt[1], qt_next, 0)
                                proj_kv(c + 1, nxt[1])
                                for s2 in (1, 2, 3):
                                    proj_qt(c + 1, nxt[1], qt_next, s2)
                    if last:
                        emit_wo(c, sts3, [0, 1, 2, 3], tail=True)
                        emit_cc(c)
                    else:
                        pending = (c, stgs, denoms8)
                        xT = nxt[1]
                        qt = qt_next

    nc.finalize()
    return nc


_NC_CACHE = None


def _get_nc():
    global _NC_CACHE
    if _NC_CACHE is None:
        _NC_CACHE = _build()
    return _NC_CACHE


def _shard_inputs(x, wq, wk, wv, wo, freqs_cos, freqs_sin):
    """Pure layout work: slice batch, pick each core's heads, permute rope
    pairs within each head, shard wo rows per core, replicate cos/sin."""
    x = np.ascontiguousarray(np.asarray(x, dtype=np.float32))
    wq = np.asarray(wq, dtype=np.float32)
    wk = np.asarray(wk, dtype=np.float32)
    wv = np.asarray(wv, dtype=np.float32)
    wo = np.asarray(wo, dtype=np.float32)
    cos = np.asarray(freqs_cos, dtype=np.float32)
    sin = np.asarray(freqs_sin, dtype=np.float32)

    # replicated rope tables matching the transposed Q^T/K^T row layout:
    # row r (within a 64-row slot block, w = r % 64, quadrant q2 = w // 16):
    # freq index i = (q2 // 2) * 16 + (w % 16); a-halves (q2 even) get -sin.
    cosr = np.empty((128, S), dtype=np.float32)
    sinr = np.empty((128, S), dtype=np.float32)
    for r in range(128):
        w = r % 64
        q2 = w // 16
        i = (q2 // 2) * 16 + (w % 16)
        cosr[r] = cos[:, i]
        sinr[r] = (-1.0 if q2 % 2 == 0 else 1.0) * sin[:, i]
    cosr = np.ascontiguousarray(cosr)
    sinr = np.ascontiguousarray(sinr)

    in_maps = []
    for core in range(N_CORES):
        b, g = core // 4, core % 4
        wq_cols = []
        wo_rows = []
        for s_ in range(8):
            h = 8 * g + SLOT_TO_LOCAL[s_]
            wq_cols.append(wq[:, 64 * h + HD_PERM])
            wo_rows.append(wo[64 * h:64 * (h + 1), :])
        wq_s = np.ascontiguousarray(np.concatenate(wq_cols, axis=1))
        wo_s = np.ascontiguousarray(np.concatenate(wo_rows, axis=0))
        wk_cols = [wk[:, 64 * (2 * g + j) + HD_PERM] for j in range(2)]
        wv_cols = wv[:, 64 * 2 * g: 64 * (2 * g + 2)]
        wkv_s = np.ascontiguousarray(
            np.concatenate(wk_cols + [wv_cols], axis=1))
        in_maps.append({
            "x": x[b], "wq": wq_s, "wkv": wkv_s, "wo": wo_s,
            "cosr": cosr, "sinr": sinr,
        })
    return in_maps


def kernel(x, wq, wk, wv, wo, freqs_cos, freqs_sin, mask=None, start_pos=0,
           **_unused):
    nc = _get_nc()
    in_maps = _shard_inputs(x, wq, wk, wv, wo, freqs_cos, freqs_sin)
    res = bass_utils.run_bass_kernel_spmd(
        nc, in_maps, core_ids=list(range(N_CORES)))
    out = np.empty((B, S, DIM), dtype=np.float32)
    for core in range(N_CORES):
        b, g = core // 4, core % 4
        co = np.asarray(res.results[core]["out"]).astype(np.float32)
        for c in range(NCH):
            out[b, CHUNK * c + 128 * g: CHUNK * c + 128 * (g + 1), :] = \
                co[128 * c:128 * (c + 1), :]
    return out
